# revision 32
# baseline (speedup 1.0000x reference)
"""Trainium2 Bass kernel for nn_CausalMoE.

Reference computation (B=2, S=2048, H=2048, G=16, GH=8, FFN=8192):
  cv        = tanh(hs @ P_extract)                        [N,G]   N = B*S = 4096
  pi        = cv @ A                                      [N,G]
  h[:,m,:]  = cv @ W1[m,:G,:] + pi[:,m,None]*W1[m,G,:] + b1[m]
  h         = gelu(h)  (exact erf gelu)                   [N,G,GH]
  effects   = sum_k h[:,m,k] W2[m,k] + b2[m]              [N,G]
  modified  = hs + 0.5 * effects @ P_route                [N,H]
  ffn_h     = gelu(modified @ ffn_w1 + ffn_b1)            [N,F]
  out       = ffn_h @ ffn_w2 + ffn_b2                     [N,H]

Strategy: pure data-parallel over the 8 NeuronCores (512 tokens/core),
weights replicated.  Everything is computed feature-major (activations
stored transposed, [feature, token]) so every matmul has its contraction
dim on partitions with weights as the stationary operand.  The host
shards hs in transposed fp16 layout and the gather transposes the output
shards back, so the kernel needs no on-chip transposes at all.  All
matmuls run in fp16: measured 216.5 ns per 512-row matmul on HW vs
227.3 ns for float32r (the PE runs 512 rows at 2.4 GHz = 213 ns floor;
f32r pays an extra ~14 ns/instr), and fp16's 10-bit mantissa is more
accurate than f32r's truncation.  fp8 DoubleRow was measured at 217 ns
per 2-k-tile instruction (2x f32r) but raw e4m3 noise is ~5e-2 rel err
(>2e-2 budget) and any compensated scheme needs >=2 slot-products per
k-tile, erasing the gain -- so fp16 is the per-core compute floor.
The tiny causal-mechanism loop is folded into two small matmuls via
host-side weight restructuring (the pi matmul is folded into W1:
W1eff = W1a + A @ W1b).  FFN runs in 4 F-blocks of 2048 with an fp16
SBUF output accumulator.  The big weights are re-tiled on the host so
every weight DMA is a single fully-contiguous 512 KiB read (4 KiB per
partition), and x/weight/output DMA bytes are all halved vs fp32.
"""
import sys

sys.path.insert(0, "/opt/trn_rl_repo")

import numpy as np

import concourse.bacc as bacc
import concourse.mybir as mybir
import concourse.tile as tile
from concourse.bass_utils import run_bass_kernel_spmd

F32 = mybir.dt.float32
F16 = mybir.dt.float16
AF = mybir.ActivationFunctionType

B, S, H = 2, 2048, 2048
G, GH, F = 16, 8, 8192
N_CORES = 8
NTOK = B * S              # 4096 tokens total
T = NTOK // N_CORES       # 512 tokens per core
KO = H // 128             # 16 contraction tiles over H
FO = F // 128             # 64 F tiles
NBLK = 4                  # F blocks
FPB = FO // NBLK          # 16 F tiles per block

_CACHE = {}


def _build():
    nc = bacc.Bacc("TRN2", target_bir_lowering=False, debug=False)
    # host-side shard layout: xtd[p, ko, t] = hs_shard.T[ko*128+p, t]
    # (partition-major so every DMA lands 128 x contiguous-KB descriptors)
    xtd = nc.dram_tensor("xtd", [128, KO, T], F16, kind="ExternalInput").ap()
    pe = nc.dram_tensor("pe", [128, KO, G], F16, kind="ExternalInput").ap()
    w1e = nc.dram_tensor("w1e", [G, G * GH], F16, kind="ExternalInput").ap()
    b1f = nc.dram_tensor("b1f", [G * GH, 1], F32, kind="ExternalInput").ap()
    w2bd = nc.dram_tensor("w2bd", [G * GH, G], F16, kind="ExternalInput").ap()
    b2s = nc.dram_tensor("b2s", [G, 1], F32, kind="ExternalInput").ap()
    pr = nc.dram_tensor("pr", [G, H], F16, kind="ExternalInput").ap()
    # pw1 = P_route @ ffn_w1  [G, F]: lets early FFN1 tiles run on RAW x
    # with a rank-16 correction, hiding the routing pass
    pw1 = nc.dram_tensor("pw1", [G, F], F16, kind="ExternalInput").ap()
    # host-retiled: fw1t[fo, p, ko, f] = ffn_w1[ko*128+p, fo*128+f]
    fw1 = nc.dram_tensor("fw1", [FO, 128, KO, 128], F16, kind="ExternalInput").ap()
    fb1 = nc.dram_tensor("fb1", [128, FO], F32, kind="ExternalInput").ap()
    # host-retiled: fw2t[ho, b, p, j, h] = ffn_w2[(b*FPB+j)*128+p, ho*128+h]
    fw2 = nc.dram_tensor(
        "fw2", [KO, NBLK, 128, FPB, 128], F16, kind="ExternalInput"
    ).ap()
    fb2 = nc.dram_tensor("fb2", [128, KO], F32, kind="ExternalInput").ap()
    # output stays feature-major, partition-major [128, KO, T] fp16; the
    # host gather untiles + transposes
    out = nc.dram_tensor("out", [128, KO, T], F16, kind="ExternalOutput").ap()

    with tile.TileContext(nc) as tc:
        with (
            tc.tile_pool(name="const", bufs=1) as const,
            tc.tile_pool(name="xt", bufs=1) as xtp,
            tc.tile_pool(name="h1", bufs=1) as h1p,
            tc.tile_pool(name="oacc", bufs=1) as oap,
            tc.tile_pool(name="w1", bufs=6) as w1p,
            tc.tile_pool(name="w2", bufs=5) as w2p,
            tc.tile_pool(name="sm", bufs=1) as smp,
            tc.tile_pool(name="mm", bufs=8, space="PSUM") as mmp,
        ):
            # explicit zero tile for activation biases: a float bias would
            # synthesize a const-AP pool whose TENSOR_LOAD sits in the
            # serialized kernel preamble (~2.7us)
            zz = const.tile([G, 1], F32)
            nc.gpsimd.memset(zz[:], 0.0)

            # PE clock warm-up: HAM keeps the PE throttled at 1.2 GHz until
            # ~3.4us of sustained matmul activity.  The PE is otherwise idle
            # while the xT shard DMAs in, so without this the extraction,
            # routing and first ~15 FFN matmuls all run at half speed.
            scr = const.tile([128, T], F16)
            nc.vector.memset(scr[:].bitcast(mybir.dt.uint16), 0)
            jp = mmp.tile([128, T], F32, tag="mm")

            def pe_keepalive(n, width=256):
                # junk matmuls that keep the PE's HAM activity window busy
                # across known dependency stalls (idle >3.4us re-throttles)
                for _ in range(n):
                    nc.tensor.matmul(
                        jp[:, 0:width], scr[:, 0:128], scr[:, 0:width],
                        start=True, stop=True,
                    )

            pe_keepalive(6, width=256)   # start tripping the un-throttle window

            # warm the ACT Tanh+Gelu LUTs during the xT load, so the
            # ~1.3us table loads are off the small-chain critical path
            act_warm = const.tile([1, 2], F32)
            nc.scalar.activation(act_warm[:, 0:1], zz[0:1, :], AF.Tanh,
                                 bias=zz[0:1, :])
            nc.scalar.activation(act_warm[:, 1:2], zz[0:1, :], AF.Gelu,
                                 bias=zz[0:1, :])

            # small consts on the gpsimd DMA queue so the sync queue is
            # free for x chunks + weight streaming from t=0
            pe_sb = const.tile([128, KO, G], F16)
            nc.gpsimd.dma_start(pe_sb[:], pe)
            w1e_sb = const.tile([G, G * GH], F16)
            nc.gpsimd.dma_start(w1e_sb[:], w1e)
            b1f_sb = const.tile([G * GH, 1], F32)
            nc.gpsimd.dma_start(b1f_sb[:], b1f)
            w2bd_sb = const.tile([G * GH, G], F16)
            nc.gpsimd.dma_start(w2bd_sb[:], w2bd)
            b2s_sb = const.tile([G, 1], F32)
            nc.gpsimd.dma_start(b2s_sb[:], b2s)
            pr_sb = const.tile([G, H], F16)
            nc.gpsimd.dma_start(pr_sb[:], pr)
            pw1_sb = const.tile([G, F], F16)
            fb1_sb = const.tile([128, FO], F32)
            nc.gpsimd.dma_start(fb1_sb[:], fb1)
            fb2_sb = const.tile([128, KO], F32)
            nc.gpsimd.dma_start(fb2_sb[:], fb2)

            # ---- load feature-major xT [128, KO, T] straight from HBM ----
            # (the host shards hs in transposed fp16 layout, so no PE
            # transposes or PSUM evictions are needed on the input side)
            # split the load across both DMA queues: sync takes ko 0-7 in
            # small groups (extraction consumes in order), gpsimd takes the
            # back half after the (small) const loads
            # pre-staged first NRAW FFN1 weight tiles (scalar queue)
            NRAW = 5
            w1pre = []
            for fo in range(NRAW):
                wtp = const.tile([128, KO, 128], F16, name=f"w1pre{fo}")
                w1pre.append(wtp)

            xT = xtp.tile([128, KO, T], F16)
            for g0, gn in [(0, 1), (1, 2), (3, 2), (5, 3)]:
                nc.sync.dma_start(
                    xT[:, g0:g0 + gn, :], xtd[:, g0:g0 + gn, :]
                )
            # back half on the scalar queue, in parallel with the front
            for g0, gn in [(8, 3), (11, 5)]:
                nc.scalar.dma_start(
                    xT[:, g0:g0 + gn, :], xtd[:, g0:g0 + gn, :]
                )
            # then the pre-staged early FFN1 weights on the scalar queue
            for fo in range(NRAW):
                nc.scalar.dma_start(w1pre[fo][:], fw1[fo])
            nc.scalar.dma_start(pw1_sb[:], pw1)

            # ---- causal-variable extraction: cv^T = tanh(Pe^T @ x^T) ----
            cv_ps = mmp.tile([128, T], F32, tag="mm")
            for ko in range(KO):
                nc.tensor.matmul(
                    cv_ps[0:G, :], pe_sb[:, ko, :], xT[:, ko, :],
                    start=(ko == 0), stop=(ko == KO - 1),
                )
                if ko < KO - 1:
                    # absorb the xT DMA pacing so the HAM activity window
                    # stays busy (idle re-throttles the PE to 1.2 GHz);
                    # the junk runs inside waits that would idle anyway
                    pe_keepalive(2, width=T)
            cvt_sb = smp.tile([G, T], F16, tag="cv")
            nc.scalar.activation(cvt_sb[:], cv_ps[0:G, :], AF.Tanh,
                                 bias=zz[:])

            # ---- FFN1: the first NRAW fo-tiles run on RAW x and add the
            # routing term via a rank-16 correction (pw1 = P_route@ffn_w1),
            # so the FFN stream starts as soon as xT has landed and the
            # whole causal chain hides behind real matmul work ----
            out_acc = oap.tile([128, KO, T], F16)
            xM = oap.tile([128, KO, T], F16, name="xM")
            h1b = h1p.tile([128, FPB, T], F16, tag="h1")

            pfs = [mmp.tile([128, T], F32, tag="mm", name=f"pfr{fo}")
                   for fo in range(NRAW)]

            def raw_xparts(fo, k0, k1):
                for ko in range(k0, k1):
                    nc.tensor.matmul(
                        pfs[fo][:], w1pre[fo][:, ko, :], xT[:, ko, :],
                        start=(ko == 0), stop=False,
                    )

            def finish_raw(fo):
                # rank-16 routing correction closes the accumulation group
                nc.tensor.matmul(
                    pfs[fo][:], pw1_sb[:, fo * 128:(fo + 1) * 128],
                    effs_sb[:], start=False, stop=True,
                )
                nc.scalar.activation(
                    h1b[:, fo, :], pfs[fo][:], AF.Gelu,
                    bias=fb1_sb[:, fo:fo + 1]
                )

            raw_xparts(0, 0, KO)

            # mechanism hidden: gelu(W1eff^T @ cv + b1); the tanh/gelu/DVE
            # latencies hide behind the fo0/fo1 matmul streams
            # (host folds the pi matmul into W1: W1eff = W1a + A @ W1b)
            h_ps = mmp.tile([128, T], F32, tag="mm")
            nc.tensor.matmul(h_ps[:], w1e_sb[:], cvt_sb[:], start=True, stop=True)
            hm_sb = smp.tile([G * GH, T], F16, tag="hm")
            nc.scalar.activation(hm_sb[:], h_ps[:], AF.Gelu, bias=b1f_sb[:])

            raw_xparts(1, 0, 8)

            # effects*0.5 = W2bd^T @ hm + b2*0.5
            eff_ps = mmp.tile([128, T], F32, tag="mm")
            nc.tensor.matmul(
                eff_ps[0:G, :], w2bd_sb[:], hm_sb[:], start=True, stop=True
            )

            raw_xparts(1, 8, KO)

            # bias-add on DVE: keeps the ACT LUT on Gelu (no table reload)
            effs_sb = smp.tile([G, T], F16, tag="eff")
            nc.vector.tensor_scalar_add(effs_sb[:], eff_ps[0:G, :], b2s_sb[:])

            finish_raw(0)
            raw_xparts(2, 0, KO)
            finish_raw(1)
            raw_xparts(3, 0, KO)
            finish_raw(2)

            # ---- modified^T = x^T + P_route^T @ effs  (into xM) ----
            # emitted mid-raw-stream: the DVE adds (~440ns each) complete
            # while fo3/fo4 still run on raw x, so fo5+ never wait on xM
            for ho in range(KO):
                md = mmp.tile([128, T], F32, tag="mm")
                nc.tensor.matmul(
                    md[:], pr_sb[:, ho * 128:(ho + 1) * 128], effs_sb[:],
                    start=True, stop=True,
                )
                nc.vector.tensor_add(xM[:, ho, :], xT[:, ho, :], md[:])

            raw_xparts(4, 0, KO)
            finish_raw(3)
            finish_raw(4)
            # bridge the last xM adds (DVE) before fo5 consumes them
            pe_keepalive(3, width=T)

            # ---- FFN in 4 F-blocks, fp16 SBUF accumulator for layer 2 ----
            out_t = out

            for b in range(NBLK):
                if b > 0:
                    h1b = h1p.tile([128, FPB, T], F16, tag="h1")
                for j in range(NRAW if b == 0 else 0, FPB):
                    fo = b * FPB + j
                    wt = w1p.tile([128, KO, 128], F16, tag="w1")
                    nc.sync.dma_start(wt[:], fw1[fo])
                    pf = mmp.tile([128, T], F32, tag="mm")
                    for ko in range(KO):
                        nc.tensor.matmul(
                            pf[:], wt[:, ko, :], xM[:, ko, :],
                            start=(ko == 0), stop=(ko == KO - 1),
                        )
                    nc.scalar.activation(
                        h1b[:, j, :], pf[:], AF.Gelu, bias=fb1_sb[:, fo:fo + 1]
                    )
                for ho in range(KO):
                    w2t = w2p.tile([128, FPB, 128], F16, tag="w2")
                    nc.sync.dma_start(w2t[:], fw2[ho, b])
                    po = mmp.tile([128, T], F32, tag="mm")
                    for j in range(FPB):
                        nc.tensor.matmul(
                            po[:], w2t[:, j, :], h1b[:, j, :],
                            start=(j == 0), stop=(j == FPB - 1),
                        )
                    if b == 0:
                        nc.vector.tensor_scalar_add(
                            out_acc[:, ho, :], po[:], fb2_sb[:, ho:ho + 1]
                        )
                    else:
                        nc.vector.tensor_add(
                            out_acc[:, ho, :], out_acc[:, ho, :], po[:]
                        )
                    if b == NBLK - 1:
                        # store this H-tile feature-major; host transposes
                        nc.sync.dma_start(out_t[:, ho, :], out_acc[:, ho, :])

    nc.compile()
    return nc


def _prep(inputs):
    """Host-side restructuring of weights + sharding."""
    hs = np.asarray(inputs["hidden_states"], np.float32)
    W1 = np.asarray(inputs["W1"], np.float32)
    b1 = np.asarray(inputs["b1"], np.float32)
    W2 = np.asarray(inputs["W2"], np.float32)
    b2 = np.asarray(inputs["b2"], np.float32)
    adj = np.asarray(inputs["causal_adjacency"], np.float32)

    w1a = W1[:, :G, :].transpose(1, 0, 2).reshape(G, G * GH)
    w1b = np.zeros((G, G * GH), np.float32)
    for m in range(G):
        w1b[m, m * GH:(m + 1) * GH] = W1[m, G, :]
    # fold the pi = cv @ A matmul into W1: h = cv @ (W1a + A @ W1b) + b1
    w1e = w1a + adj @ w1b
    b1f = b1.reshape(G * GH, 1)
    w2bd = np.zeros((G * GH, G), np.float32)
    for m in range(G):
        w2bd[m * GH:(m + 1) * GH, m] = 0.5 * W2[m, :]
    b2s = (0.5 * b2).reshape(G, 1)

    pe = np.asarray(inputs["P_extract"], np.float32)
    # pe[h, g] -> [p, ko, g] with h = ko*128 + p
    pe_t = np.ascontiguousarray(
        pe.reshape(KO, 128, G).transpose(1, 0, 2).astype(np.float16)
    )

    fw1 = np.asarray(inputs["ffn_w1"], np.float32)
    # fw1[ko*128+p, fo*128+f] -> [fo, p, ko, f]
    fw1_t = np.ascontiguousarray(
        fw1.reshape(KO, 128, FO, 128).transpose(2, 1, 0, 3).astype(np.float16)
    )
    fw2 = np.asarray(inputs["ffn_w2"], np.float32)
    # fw2[(b*FPB+j)*128+p, ho*128+h] -> [ho, b, p, j, h]
    fw2_t = np.ascontiguousarray(
        fw2.reshape(NBLK, FPB, 128, KO, 128).transpose(3, 0, 2, 1, 4)
        .astype(np.float16)
    )

    common = {
        "pe": pe_t,
        "w1e": np.ascontiguousarray(w1e.astype(np.float16)),
        "b1f": np.ascontiguousarray(b1f),
        "w2bd": np.ascontiguousarray(w2bd.astype(np.float16)),
        "b2s": np.ascontiguousarray(b2s),
        "pr": np.ascontiguousarray(
            np.asarray(inputs["P_route"], np.float32).astype(np.float16)
        ),
        # rank-16 routing correction for the raw-x FFN1 tiles; the 0.5
        # intervention strength is already folded into effs via w2bd/b2s
        "pw1": np.ascontiguousarray(
            (np.asarray(inputs["P_route"], np.float32)
             @ np.asarray(inputs["ffn_w1"], np.float32)).astype(np.float16)
        ),
        "fw1": fw1_t,
        "fb1": np.ascontiguousarray(
            np.asarray(inputs["ffn_b1"], np.float32).reshape(FO, 128).T
        ),
        "fw2": fw2_t,
        "fb2": np.ascontiguousarray(
            np.asarray(inputs["ffn_b2"], np.float32).reshape(KO, 128).T
        ),
    }
    toks = hs.reshape(NTOK, H)
    in_maps = []
    for c in range(N_CORES):
        m = dict(common)
        # [T, H] -> transpose -> [H, T] -> [KO, 128, T] -> [128, KO, T]
        m["xtd"] = np.ascontiguousarray(
            toks[c * T:(c + 1) * T].T.reshape(KO, 128, T).transpose(1, 0, 2)
            .astype(np.float16)
        )
        in_maps.append(m)
    return in_maps


def run(inputs, trace=False):
    """Returns (full output [B,S,H] fp32, BassKernelResults)."""
    if "nc" not in _CACHE:
        _CACHE["nc"] = _build()
    nc = _CACHE["nc"]
    in_maps = _prep(inputs)
    res = run_bass_kernel_spmd(
        nc, in_maps, core_ids=list(range(N_CORES)), trace=trace
    )
    full = np.empty((NTOK, H), np.float32)
    for c in range(N_CORES):
        # [128, KO, T] -> [KO, 128, T] = [H, T] -> [T, H]
        o = res.results[c]["out"].transpose(1, 0, 2).reshape(H, T)
        full[c * T:(c + 1) * T] = o.T.astype(np.float32)
    return full.reshape(B, S, H), res


def kernel(**inputs):
    full, _ = run(inputs, trace=False)
    return full


# revision 33
# speedup vs baseline: 1.0146x; 1.0146x over previous
"""Trainium2 Bass kernel for nn_CausalMoE.

Reference computation (B=2, S=2048, H=2048, G=16, GH=8, FFN=8192):
  cv        = tanh(hs @ P_extract)                        [N,G]   N = B*S = 4096
  pi        = cv @ A                                      [N,G]
  h[:,m,:]  = cv @ W1[m,:G,:] + pi[:,m,None]*W1[m,G,:] + b1[m]
  h         = gelu(h)  (exact erf gelu)                   [N,G,GH]
  effects   = sum_k h[:,m,k] W2[m,k] + b2[m]              [N,G]
  modified  = hs + 0.5 * effects @ P_route                [N,H]
  ffn_h     = gelu(modified @ ffn_w1 + ffn_b1)            [N,F]
  out       = ffn_h @ ffn_w2 + ffn_b2                     [N,H]

Strategy: pure data-parallel over the 8 NeuronCores (512 tokens/core),
weights replicated.  Everything is computed feature-major (activations
stored transposed, [feature, token]) so every matmul has its contraction
dim on partitions with weights as the stationary operand.  The host
shards hs in transposed, partition-major fp16 layout and the gather
untiles/transposes the output shards back, so the kernel needs no
on-chip transposes at all.  All matmuls run in fp16: measured 216.5 ns
per 512-row matmul on HW vs 227.3 ns for float32r (the PE runs 512 rows
at 2.4 GHz = 213 ns floor; f32r pays an extra ~14 ns/instr), and fp16's
10-bit mantissa is more accurate than f32r's truncation.  fp8 DoubleRow
was measured at 217 ns per 2-k-tile instruction (2x f32r MACs) but raw
e4m3 noise is ~5e-2 rel err (>2e-2 budget) and any hi-lo compensated
scheme needs >=3 slot-products per 2 k-tiles, erasing the gain -- so
fp16 is the per-core compute floor (~444 us for the 2048 FFN matmuls).
The tiny causal-mechanism loop is folded into two small matmuls via
host-side weight restructuring (the pi matmul is folded into W1:
W1eff = W1a + A @ W1b).  FFN runs in 4 F-blocks of 2048 with an fp16
SBUF output accumulator.  The big weights are re-tiled on the host so
every weight DMA is a single fully-contiguous 512 KiB read (4 KiB per
partition), and x/weight/output DMA bytes are all halved vs fp32.
Junk keepalive matmuls bridge the xT-DMA and activation-latency waits
in the pre-FFN chain: the HAM power manager re-throttles the PE to
1.2 GHz after sub-us idles, which would otherwise slow the routing and
first ~25 FFN matmuls by ~2x.
"""
import sys

sys.path.insert(0, "/opt/trn_rl_repo")

import numpy as np

import concourse.bacc as bacc
import concourse.mybir as mybir
import concourse.tile as tile
from concourse.bass_utils import run_bass_kernel_spmd

F32 = mybir.dt.float32
F16 = mybir.dt.float16
AF = mybir.ActivationFunctionType

B, S, H = 2, 2048, 2048
G, GH, F = 16, 8, 8192
N_CORES = 8
NTOK = B * S              # 4096 tokens total
T = NTOK // N_CORES       # 512 tokens per core
KO = H // 128             # 16 contraction tiles over H
FO = F // 128             # 64 F tiles
NBLK = 4                  # F blocks
FPB = FO // NBLK          # 16 F tiles per block

_CACHE = {}


def _build():
    nc = bacc.Bacc("TRN2", target_bir_lowering=False, debug=False)
    # host-side shard layout: xtd[p, ko, t] = hs_shard.T[ko*128+p, t]
    # (partition-major so every DMA lands 128 x contiguous-KB descriptors)
    xtd = nc.dram_tensor("xtd", [128, KO, T], F16, kind="ExternalInput").ap()
    pe = nc.dram_tensor("pe", [128, KO, G], F16, kind="ExternalInput").ap()
    w1e = nc.dram_tensor("w1e", [G, G * GH], F16, kind="ExternalInput").ap()
    b1f = nc.dram_tensor("b1f", [G * GH, 1], F32, kind="ExternalInput").ap()
    w2bd = nc.dram_tensor("w2bd", [G * GH, G], F16, kind="ExternalInput").ap()
    b2s = nc.dram_tensor("b2s", [G, 1], F32, kind="ExternalInput").ap()
    pr = nc.dram_tensor("pr", [G, H], F16, kind="ExternalInput").ap()
    # host-retiled: fw1t[fo, p, ko, f] = ffn_w1[ko*128+p, fo*128+f]
    fw1 = nc.dram_tensor("fw1", [FO, 128, KO, 128], F16, kind="ExternalInput").ap()
    fb1 = nc.dram_tensor("fb1", [128, FO], F32, kind="ExternalInput").ap()
    # host-retiled: fw2t[ho, b, p, j, h] = ffn_w2[(b*FPB+j)*128+p, ho*128+h]
    fw2 = nc.dram_tensor(
        "fw2", [KO, NBLK, 128, FPB, 128], F16, kind="ExternalInput"
    ).ap()
    fb2 = nc.dram_tensor("fb2", [128, KO], F32, kind="ExternalInput").ap()
    # output stays feature-major, partition-major [128, KO, T] fp16; the
    # host gather untiles + transposes
    out = nc.dram_tensor("out", [128, KO, T], F16, kind="ExternalOutput").ap()

    with tile.TileContext(nc) as tc:
        with (
            tc.tile_pool(name="const", bufs=1) as const,
            tc.tile_pool(name="xt", bufs=1) as xtp,
            tc.tile_pool(name="h1", bufs=1) as h1p,
            tc.tile_pool(name="oacc", bufs=1) as oap,
            tc.tile_pool(name="w1", bufs=6) as w1p,
            tc.tile_pool(name="w2", bufs=5) as w2p,
            tc.tile_pool(name="sm", bufs=1) as smp,
            tc.tile_pool(name="mm", bufs=6, space="PSUM") as mmp,
        ):
            # explicit zero tile for activation biases: a float bias would
            # synthesize a const-AP pool whose TENSOR_LOAD sits in the
            # serialized kernel preamble (~2.7us)
            zz = const.tile([G, 1], F32)
            nc.gpsimd.memset(zz[:], 0.0)

            # PE clock warm-up: HAM keeps the PE throttled at 1.2 GHz until
            # ~3.4us of sustained matmul activity.  The PE is otherwise idle
            # while the xT shard DMAs in, so without this the extraction,
            # routing and first ~15 FFN matmuls all run at half speed.
            scr = const.tile([128, T], F16)
            nc.vector.memset(scr[:].bitcast(mybir.dt.uint16), 0)
            jp = mmp.tile([128, T], F32, tag="mm")

            def pe_keepalive(n, width=256):
                # junk matmuls that keep the PE's HAM activity window busy
                # across known dependency stalls (idle re-throttles)
                for _ in range(n):
                    nc.tensor.matmul(
                        jp[:, 0:width], scr[:, 0:128], scr[:, 0:width],
                        start=True, stop=True,
                    )

            pe_keepalive(8, width=256)   # trip the un-throttle window

            # warm the ACT Tanh+Gelu LUTs during the xT load, so the
            # ~1.3us table loads are off the small-chain critical path
            act_warm = const.tile([1, 2], F32)
            nc.scalar.activation(act_warm[:, 0:1], zz[0:1, :], AF.Tanh,
                                 bias=zz[0:1, :])
            nc.scalar.activation(act_warm[:, 1:2], zz[0:1, :], AF.Gelu,
                                 bias=zz[0:1, :])

            # small consts on the gpsimd DMA queue so the sync queue is
            # free for x chunks + weight streaming from t=0
            pe_sb = const.tile([128, KO, G], F16)
            nc.gpsimd.dma_start(pe_sb[:], pe)
            w1e_sb = const.tile([G, G * GH], F16)
            nc.gpsimd.dma_start(w1e_sb[:], w1e)
            b1f_sb = const.tile([G * GH, 1], F32)
            nc.gpsimd.dma_start(b1f_sb[:], b1f)
            w2bd_sb = const.tile([G * GH, G], F16)
            nc.gpsimd.dma_start(w2bd_sb[:], w2bd)
            b2s_sb = const.tile([G, 1], F32)
            nc.gpsimd.dma_start(b2s_sb[:], b2s)
            pr_sb = const.tile([G, H], F16)
            nc.gpsimd.dma_start(pr_sb[:], pr)
            fb1_sb = const.tile([128, FO], F32)
            nc.gpsimd.dma_start(fb1_sb[:], fb1)
            fb2_sb = const.tile([128, KO], F32)
            nc.gpsimd.dma_start(fb2_sb[:], fb2)

            # ---- load feature-major xT [128, KO, T] straight from HBM ----
            # (the host shards hs in transposed fp16 layout, so no PE
            # transposes or PSUM evictions are needed on the input side)
            xT = xtp.tile([128, KO, T], F16)
            for g0, gn in [(0, 1), (1, 2), (3, 3), (6, 5), (11, 5)]:
                nc.sync.dma_start(
                    xT[:, g0:g0 + gn, :], xtd[:, g0:g0 + gn, :]
                )

            # ---- causal-variable extraction: cv^T = tanh(Pe^T @ x^T) ----
            cv_ps = mmp.tile([128, T], F32, tag="mm")
            for ko in range(KO):
                nc.tensor.matmul(
                    cv_ps[0:G, :], pe_sb[:, ko, :], xT[:, ko, :],
                    start=(ko == 0), stop=(ko == KO - 1),
                )
                if ko in (2, 5, 8, 11):
                    # absorb the wait for the next xT DMA group so the
                    # HAM activity window stays busy
                    pe_keepalive(3, width=T)
            cvt_sb = smp.tile([G, T], F16, tag="cv")
            nc.scalar.activation(cvt_sb[:], cv_ps[0:G, :], AF.Tanh,
                                 bias=zz[:])
            pe_keepalive(7, width=T)

            # mechanism hidden: gelu(W1eff^T @ cv + b1)
            # (host folds the pi matmul into W1: W1eff = W1a + A @ W1b)
            h_ps = mmp.tile([128, T], F32, tag="mm")
            nc.tensor.matmul(h_ps[:], w1e_sb[:], cvt_sb[:], start=True, stop=True)
            hm_sb = smp.tile([G * GH, T], F16, tag="hm")
            nc.scalar.activation(hm_sb[:], h_ps[:], AF.Gelu, bias=b1f_sb[:])
            pe_keepalive(7, width=T)

            # effects*0.5 = W2bd^T @ hm + b2*0.5
            eff_ps = mmp.tile([128, T], F32, tag="mm")
            nc.tensor.matmul(
                eff_ps[0:G, :], w2bd_sb[:], hm_sb[:], start=True, stop=True
            )
            # bias-add on DVE: keeps the ACT LUT on Gelu (no table reload)
            effs_sb = smp.tile([G, T], F16, tag="eff")
            nc.vector.tensor_scalar_add(effs_sb[:], eff_ps[0:G, :], b2s_sb[:])
            pe_keepalive(7, width=T)

            # ---- modified^T = x^T + P_route^T @ effs  (in place on xT) ----
            for ho in range(KO):
                md = mmp.tile([128, T], F32, tag="mm")
                nc.tensor.matmul(
                    md[:], pr_sb[:, ho * 128:(ho + 1) * 128], effs_sb[:],
                    start=True, stop=True,
                )
                nc.vector.tensor_add(xT[:, ho, :], xT[:, ho, :], md[:])

            # ---- FFN in 4 F-blocks, fp16 SBUF accumulator for layer 2 ----
            out_acc = oap.tile([128, KO, T], F16)

            out_t = out

            for b in range(NBLK):
                h1b = h1p.tile([128, FPB, T], F16, tag="h1")
                for j in range(FPB):
                    fo = b * FPB + j
                    wt = w1p.tile([128, KO, 128], F16, tag="w1")
                    nc.sync.dma_start(wt[:], fw1[fo])
                    pf = mmp.tile([128, T], F32, tag="mm")
                    for ko in range(KO):
                        nc.tensor.matmul(
                            pf[:], wt[:, ko, :], xT[:, ko, :],
                            start=(ko == 0), stop=(ko == KO - 1),
                        )
                    nc.scalar.activation(
                        h1b[:, j, :], pf[:], AF.Gelu, bias=fb1_sb[:, fo:fo + 1]
                    )
                for ho in range(KO):
                    w2t = w2p.tile([128, FPB, 128], F16, tag="w2")
                    nc.sync.dma_start(w2t[:], fw2[ho, b])
                    po = mmp.tile([128, T], F32, tag="mm")
                    for j in range(FPB):
                        nc.tensor.matmul(
                            po[:], w2t[:, j, :], h1b[:, j, :],
                            start=(j == 0), stop=(j == FPB - 1),
                        )
                    if b == 0:
                        nc.vector.tensor_scalar_add(
                            out_acc[:, ho, :], po[:], fb2_sb[:, ho:ho + 1]
                        )
                    else:
                        nc.vector.tensor_add(
                            out_acc[:, ho, :], out_acc[:, ho, :], po[:]
                        )
                    if b == NBLK - 1:
                        # store this H-tile feature-major; host transposes
                        nc.sync.dma_start(out_t[:, ho, :], out_acc[:, ho, :])

    nc.compile()
    return nc


def _prep(inputs):
    """Host-side restructuring of weights + sharding."""
    hs = np.asarray(inputs["hidden_states"], np.float32)
    W1 = np.asarray(inputs["W1"], np.float32)
    b1 = np.asarray(inputs["b1"], np.float32)
    W2 = np.asarray(inputs["W2"], np.float32)
    b2 = np.asarray(inputs["b2"], np.float32)
    adj = np.asarray(inputs["causal_adjacency"], np.float32)

    w1a = W1[:, :G, :].transpose(1, 0, 2).reshape(G, G * GH)
    w1b = np.zeros((G, G * GH), np.float32)
    for m in range(G):
        w1b[m, m * GH:(m + 1) * GH] = W1[m, G, :]
    # fold the pi = cv @ A matmul into W1: h = cv @ (W1a + A @ W1b) + b1
    w1e = w1a + adj @ w1b
    b1f = b1.reshape(G * GH, 1)
    w2bd = np.zeros((G * GH, G), np.float32)
    for m in range(G):
        w2bd[m * GH:(m + 1) * GH, m] = 0.5 * W2[m, :]
    b2s = (0.5 * b2).reshape(G, 1)

    pe = np.asarray(inputs["P_extract"], np.float32)
    # pe[h, g] -> [p, ko, g] with h = ko*128 + p
    pe_t = np.ascontiguousarray(
        pe.reshape(KO, 128, G).transpose(1, 0, 2).astype(np.float16)
    )

    fw1 = np.asarray(inputs["ffn_w1"], np.float32)
    # fw1[ko*128+p, fo*128+f] -> [fo, p, ko, f]
    fw1_t = np.ascontiguousarray(
        fw1.reshape(KO, 128, FO, 128).transpose(2, 1, 0, 3).astype(np.float16)
    )
    fw2 = np.asarray(inputs["ffn_w2"], np.float32)
    # fw2[(b*FPB+j)*128+p, ho*128+h] -> [ho, b, p, j, h]
    fw2_t = np.ascontiguousarray(
        fw2.reshape(NBLK, FPB, 128, KO, 128).transpose(3, 0, 2, 1, 4)
        .astype(np.float16)
    )

    common = {
        "pe": pe_t,
        "w1e": np.ascontiguousarray(w1e.astype(np.float16)),
        "b1f": np.ascontiguousarray(b1f),
        "w2bd": np.ascontiguousarray(w2bd.astype(np.float16)),
        "b2s": np.ascontiguousarray(b2s),
        "pr": np.ascontiguousarray(
            np.asarray(inputs["P_route"], np.float32).astype(np.float16)
        ),
        "fw1": fw1_t,
        "fb1": np.ascontiguousarray(
            np.asarray(inputs["ffn_b1"], np.float32).reshape(FO, 128).T
        ),
        "fw2": fw2_t,
        "fb2": np.ascontiguousarray(
            np.asarray(inputs["ffn_b2"], np.float32).reshape(KO, 128).T
        ),
    }
    toks = hs.reshape(NTOK, H)
    in_maps = []
    for c in range(N_CORES):
        m = dict(common)
        # [T, H] -> transpose -> [H, T] -> [KO, 128, T] -> [128, KO, T]
        m["xtd"] = np.ascontiguousarray(
            toks[c * T:(c + 1) * T].T.reshape(KO, 128, T).transpose(1, 0, 2)
            .astype(np.float16)
        )
        in_maps.append(m)
    return in_maps


def run(inputs, trace=False):
    """Returns (full output [B,S,H] fp32, BassKernelResults)."""
    if "nc" not in _CACHE:
        _CACHE["nc"] = _build()
    nc = _CACHE["nc"]
    in_maps = _prep(inputs)
    res = run_bass_kernel_spmd(
        nc, in_maps, core_ids=list(range(N_CORES)), trace=trace
    )
    full = np.empty((NTOK, H), np.float32)
    for c in range(N_CORES):
        # [128, KO, T] -> [KO, 128, T] = [H, T] -> [T, H]
        o = res.results[c]["out"].transpose(1, 0, 2).reshape(H, T)
        full[c * T:(c + 1) * T] = o.T.astype(np.float32)
    return full.reshape(B, S, H), res


def kernel(**inputs):
    full, _ = run(inputs, trace=False)
    return full


# revision 34
# speedup vs baseline: 1.0150x; 1.0004x over previous
"""Trainium2 Bass kernel for nn_CausalMoE.

Reference computation (B=2, S=2048, H=2048, G=16, GH=8, FFN=8192):
  cv        = tanh(hs @ P_extract)                        [N,G]   N = B*S = 4096
  pi        = cv @ A                                      [N,G]
  h[:,m,:]  = cv @ W1[m,:G,:] + pi[:,m,None]*W1[m,G,:] + b1[m]
  h         = gelu(h)  (exact erf gelu)                   [N,G,GH]
  effects   = sum_k h[:,m,k] W2[m,k] + b2[m]              [N,G]
  modified  = hs + 0.5 * effects @ P_route                [N,H]
  ffn_h     = gelu(modified @ ffn_w1 + ffn_b1)            [N,F]
  out       = ffn_h @ ffn_w2 + ffn_b2                     [N,H]

Strategy: pure data-parallel over the 8 NeuronCores (512 tokens/core),
weights replicated.  Everything is computed feature-major (activations
stored transposed, [feature, token]) so every matmul has its contraction
dim on partitions with weights as the stationary operand.  The host
shards hs in transposed, partition-major fp16 layout and the gather
untiles/transposes the output shards back, so the kernel needs no
on-chip transposes at all.  All matmuls run in fp16: measured 216.5 ns
per 512-row matmul on HW vs 227.3 ns for float32r (the PE runs 512 rows
at 2.4 GHz = 213 ns floor; f32r pays an extra ~14 ns/instr), and fp16's
10-bit mantissa is more accurate than f32r's truncation.  fp8 DoubleRow
was measured at 217 ns per 2-k-tile instruction (2x f32r MACs) but raw
e4m3 noise is ~5e-2 rel err (>2e-2 budget) and any hi-lo compensated
scheme needs >=3 slot-products per 2 k-tiles, erasing the gain -- so
fp16 is the per-core compute floor (~444 us for the 2048 FFN matmuls).
The tiny causal-mechanism loop is folded into two small matmuls via
host-side weight restructuring (the pi matmul is folded into W1:
W1eff = W1a + A @ W1b).  FFN runs in 4 F-blocks of 2048 with an fp16
SBUF output accumulator.  The big weights are re-tiled on the host so
every weight DMA is a single fully-contiguous 512 KiB read (4 KiB per
partition), and x/weight/output DMA bytes are all halved vs fp32.
Junk keepalive matmuls bridge the xT-DMA and activation-latency waits
in the pre-FFN chain: the HAM power manager re-throttles the PE to
1.2 GHz after sub-us idles, which would otherwise slow the routing and
first ~25 FFN matmuls by ~2x.
"""
import sys

sys.path.insert(0, "/opt/trn_rl_repo")

import numpy as np

import concourse.bacc as bacc
import concourse.mybir as mybir
import concourse.tile as tile
from concourse.bass_utils import run_bass_kernel_spmd

F32 = mybir.dt.float32
F16 = mybir.dt.float16
AF = mybir.ActivationFunctionType

B, S, H = 2, 2048, 2048
G, GH, F = 16, 8, 8192
N_CORES = 8
NTOK = B * S              # 4096 tokens total
T = NTOK // N_CORES       # 512 tokens per core
KO = H // 128             # 16 contraction tiles over H
FO = F // 128             # 64 F tiles
NBLK = 4                  # F blocks
FPB = FO // NBLK          # 16 F tiles per block

_CACHE = {}


def _build():
    nc = bacc.Bacc("TRN2", target_bir_lowering=False, debug=False)
    # host-side shard layout: xtd[p, ko, t] = hs_shard.T[ko*128+p, t]
    # (partition-major so every DMA lands 128 x contiguous-KB descriptors)
    xtd = nc.dram_tensor("xtd", [128, KO, T], F16, kind="ExternalInput").ap()
    pe = nc.dram_tensor("pe", [128, KO, G], F16, kind="ExternalInput").ap()
    w1e = nc.dram_tensor("w1e", [G, G * GH], F16, kind="ExternalInput").ap()
    b1f = nc.dram_tensor("b1f", [G * GH, 1], F32, kind="ExternalInput").ap()
    w2bd = nc.dram_tensor("w2bd", [G * GH, G], F16, kind="ExternalInput").ap()
    b2s = nc.dram_tensor("b2s", [G, 1], F32, kind="ExternalInput").ap()
    pr = nc.dram_tensor("pr", [G, H], F16, kind="ExternalInput").ap()
    # host-retiled: fw1t[fo, p, ko, f] = ffn_w1[ko*128+p, fo*128+f]
    fw1 = nc.dram_tensor("fw1", [FO, 128, KO, 128], F16, kind="ExternalInput").ap()
    fb1 = nc.dram_tensor("fb1", [128, FO], F32, kind="ExternalInput").ap()
    # host-retiled: fw2t[ho, b, p, j, h] = ffn_w2[(b*FPB+j)*128+p, ho*128+h]
    fw2 = nc.dram_tensor(
        "fw2", [KO, NBLK, 128, FPB, 128], F16, kind="ExternalInput"
    ).ap()
    fb2 = nc.dram_tensor("fb2", [128, KO], F32, kind="ExternalInput").ap()
    # output stays feature-major, partition-major [128, KO, T] fp16; the
    # host gather untiles + transposes
    out = nc.dram_tensor("out", [128, KO, T], F16, kind="ExternalOutput").ap()

    with tile.TileContext(nc) as tc:
        with (
            tc.tile_pool(name="const", bufs=1) as const,
            tc.tile_pool(name="xt", bufs=1) as xtp,
            tc.tile_pool(name="h1", bufs=1) as h1p,
            tc.tile_pool(name="oacc", bufs=1) as oap,
            tc.tile_pool(name="w1", bufs=6) as w1p,
            tc.tile_pool(name="w2", bufs=5) as w2p,
            tc.tile_pool(name="sm", bufs=1) as smp,
            tc.tile_pool(name="mm", bufs=6, space="PSUM") as mmp,
        ):
            # explicit zero tile for activation biases: a float bias would
            # synthesize a const-AP pool whose TENSOR_LOAD sits in the
            # serialized kernel preamble (~2.7us)
            zz = const.tile([G, 1], F32)
            nc.gpsimd.memset(zz[:], 0.0)

            # PE clock warm-up: HAM keeps the PE throttled at 1.2 GHz until
            # ~3.4us of sustained matmul activity.  The PE is otherwise idle
            # while the xT shard DMAs in, so without this the extraction,
            # routing and first ~15 FFN matmuls all run at half speed.
            scr = const.tile([128, T], F16)
            nc.vector.memset(scr[:].bitcast(mybir.dt.uint16), 0)
            jp = mmp.tile([128, T], F32, tag="mm")

            def pe_keepalive(n, width=256):
                # junk matmuls that keep the PE's HAM activity window busy
                # across known dependency stalls (idle re-throttles)
                for _ in range(n):
                    nc.tensor.matmul(
                        jp[:, 0:width], scr[:, 0:128], scr[:, 0:width],
                        start=True, stop=True,
                    )

            pe_keepalive(8, width=256)   # trip the un-throttle window

            # warm the ACT Tanh+Gelu LUTs during the xT load, so the
            # ~1.3us table loads are off the small-chain critical path
            act_warm = const.tile([1, 2], F32)
            nc.scalar.activation(act_warm[:, 0:1], zz[0:1, :], AF.Tanh,
                                 bias=zz[0:1, :])
            nc.scalar.activation(act_warm[:, 1:2], zz[0:1, :], AF.Gelu,
                                 bias=zz[0:1, :])

            # small consts on the gpsimd DMA queue so the sync queue is
            # free for x chunks + weight streaming from t=0
            pe_sb = const.tile([128, KO, G], F16)
            nc.gpsimd.dma_start(pe_sb[:], pe)
            w1e_sb = const.tile([G, G * GH], F16)
            nc.gpsimd.dma_start(w1e_sb[:], w1e)
            b1f_sb = const.tile([G * GH, 1], F32)
            nc.gpsimd.dma_start(b1f_sb[:], b1f)
            w2bd_sb = const.tile([G * GH, G], F16)
            nc.gpsimd.dma_start(w2bd_sb[:], w2bd)
            b2s_sb = const.tile([G, 1], F32)
            nc.gpsimd.dma_start(b2s_sb[:], b2s)
            pr_sb = const.tile([G, H], F16)
            nc.gpsimd.dma_start(pr_sb[:], pr)
            fb1_sb = const.tile([128, FO], F32)
            nc.gpsimd.dma_start(fb1_sb[:], fb1)
            fb2_sb = const.tile([128, KO], F32)
            nc.gpsimd.dma_start(fb2_sb[:], fb2)

            # ---- load feature-major xT [128, KO, T] straight from HBM ----
            # (the host shards hs in transposed fp16 layout, so no PE
            # transposes or PSUM evictions are needed on the input side)
            xT = xtp.tile([128, KO, T], F16)
            for g0, gn in [(0, 1), (1, 2), (3, 3), (6, 5), (11, 5)]:
                nc.sync.dma_start(
                    xT[:, g0:g0 + gn, :], xtd[:, g0:g0 + gn, :]
                )

            # ---- causal-variable extraction: cv^T = tanh(Pe^T @ x^T) ----
            cv_ps = mmp.tile([128, T], F32, tag="mm")
            for ko in range(KO):
                nc.tensor.matmul(
                    cv_ps[0:G, :], pe_sb[:, ko, :], xT[:, ko, :],
                    start=(ko == 0), stop=(ko == KO - 1),
                )
                if ko in (2, 5, 8, 11):
                    # absorb the wait for the next xT DMA group so the
                    # HAM activity window stays busy
                    pe_keepalive(3, width=T)
            cvt_sb = smp.tile([G, T], F16, tag="cv")
            nc.scalar.activation(cvt_sb[:], cv_ps[0:G, :], AF.Tanh,
                                 bias=zz[:])
            pe_keepalive(4, width=T)

            # mechanism hidden: gelu(W1eff^T @ cv + b1)
            # (host folds the pi matmul into W1: W1eff = W1a + A @ W1b)
            h_ps = mmp.tile([128, T], F32, tag="mm")
            nc.tensor.matmul(h_ps[:], w1e_sb[:], cvt_sb[:], start=True, stop=True)
            hm_sb = smp.tile([G * GH, T], F16, tag="hm")
            nc.scalar.activation(hm_sb[:], h_ps[:], AF.Gelu, bias=b1f_sb[:])
            pe_keepalive(4, width=T)

            # effects*0.5 = W2bd^T @ hm + b2*0.5
            eff_ps = mmp.tile([128, T], F32, tag="mm")
            nc.tensor.matmul(
                eff_ps[0:G, :], w2bd_sb[:], hm_sb[:], start=True, stop=True
            )
            # bias-add on DVE: keeps the ACT LUT on Gelu (no table reload)
            effs_sb = smp.tile([G, T], F16, tag="eff")
            nc.vector.tensor_scalar_add(effs_sb[:], eff_ps[0:G, :], b2s_sb[:])
            pe_keepalive(5, width=T)

            # ---- modified^T = x^T + P_route^T @ effs  (in place on xT) ----
            for ho in range(KO):
                md = mmp.tile([128, T], F32, tag="mm")
                nc.tensor.matmul(
                    md[:], pr_sb[:, ho * 128:(ho + 1) * 128], effs_sb[:],
                    start=True, stop=True,
                )
                nc.vector.tensor_add(xT[:, ho, :], xT[:, ho, :], md[:])

            # ---- FFN in 4 F-blocks, fp16 SBUF accumulator for layer 2 ----
            out_acc = oap.tile([128, KO, T], F16)

            out_t = out

            for b in range(NBLK):
                h1b = h1p.tile([128, FPB, T], F16, tag="h1")
                for j in range(FPB):
                    fo = b * FPB + j
                    wt = w1p.tile([128, KO, 128], F16, tag="w1")
                    nc.sync.dma_start(wt[:], fw1[fo])
                    pf = mmp.tile([128, T], F32, tag="mm")
                    for ko in range(KO):
                        nc.tensor.matmul(
                            pf[:], wt[:, ko, :], xT[:, ko, :],
                            start=(ko == 0), stop=(ko == KO - 1),
                        )
                    nc.scalar.activation(
                        h1b[:, j, :], pf[:], AF.Gelu, bias=fb1_sb[:, fo:fo + 1]
                    )
                for ho in range(KO):
                    w2t = w2p.tile([128, FPB, 128], F16, tag="w2")
                    nc.sync.dma_start(w2t[:], fw2[ho, b])
                    po = mmp.tile([128, T], F32, tag="mm")
                    for j in range(FPB):
                        nc.tensor.matmul(
                            po[:], w2t[:, j, :], h1b[:, j, :],
                            start=(j == 0), stop=(j == FPB - 1),
                        )
                    if b == 0:
                        nc.vector.tensor_scalar_add(
                            out_acc[:, ho, :], po[:], fb2_sb[:, ho:ho + 1]
                        )
                    else:
                        nc.vector.tensor_add(
                            out_acc[:, ho, :], out_acc[:, ho, :], po[:]
                        )
                    if b == NBLK - 1:
                        # store this H-tile feature-major; host transposes
                        nc.sync.dma_start(out_t[:, ho, :], out_acc[:, ho, :])

    nc.compile()
    return nc


def _prep(inputs):
    """Host-side restructuring of weights + sharding."""
    hs = np.asarray(inputs["hidden_states"], np.float32)
    W1 = np.asarray(inputs["W1"], np.float32)
    b1 = np.asarray(inputs["b1"], np.float32)
    W2 = np.asarray(inputs["W2"], np.float32)
    b2 = np.asarray(inputs["b2"], np.float32)
    adj = np.asarray(inputs["causal_adjacency"], np.float32)

    w1a = W1[:, :G, :].transpose(1, 0, 2).reshape(G, G * GH)
    w1b = np.zeros((G, G * GH), np.float32)
    for m in range(G):
        w1b[m, m * GH:(m + 1) * GH] = W1[m, G, :]
    # fold the pi = cv @ A matmul into W1: h = cv @ (W1a + A @ W1b) + b1
    w1e = w1a + adj @ w1b
    b1f = b1.reshape(G * GH, 1)
    w2bd = np.zeros((G * GH, G), np.float32)
    for m in range(G):
        w2bd[m * GH:(m + 1) * GH, m] = 0.5 * W2[m, :]
    b2s = (0.5 * b2).reshape(G, 1)

    pe = np.asarray(inputs["P_extract"], np.float32)
    # pe[h, g] -> [p, ko, g] with h = ko*128 + p
    pe_t = np.ascontiguousarray(
        pe.reshape(KO, 128, G).transpose(1, 0, 2).astype(np.float16)
    )

    fw1 = np.asarray(inputs["ffn_w1"], np.float32)
    # fw1[ko*128+p, fo*128+f] -> [fo, p, ko, f]
    fw1_t = np.ascontiguousarray(
        fw1.reshape(KO, 128, FO, 128).transpose(2, 1, 0, 3).astype(np.float16)
    )
    fw2 = np.asarray(inputs["ffn_w2"], np.float32)
    # fw2[(b*FPB+j)*128+p, ho*128+h] -> [ho, b, p, j, h]
    fw2_t = np.ascontiguousarray(
        fw2.reshape(NBLK, FPB, 128, KO, 128).transpose(3, 0, 2, 1, 4)
        .astype(np.float16)
    )

    common = {
        "pe": pe_t,
        "w1e": np.ascontiguousarray(w1e.astype(np.float16)),
        "b1f": np.ascontiguousarray(b1f),
        "w2bd": np.ascontiguousarray(w2bd.astype(np.float16)),
        "b2s": np.ascontiguousarray(b2s),
        "pr": np.ascontiguousarray(
            np.asarray(inputs["P_route"], np.float32).astype(np.float16)
        ),
        "fw1": fw1_t,
        "fb1": np.ascontiguousarray(
            np.asarray(inputs["ffn_b1"], np.float32).reshape(FO, 128).T
        ),
        "fw2": fw2_t,
        "fb2": np.ascontiguousarray(
            np.asarray(inputs["ffn_b2"], np.float32).reshape(KO, 128).T
        ),
    }
    toks = hs.reshape(NTOK, H)
    in_maps = []
    for c in range(N_CORES):
        m = dict(common)
        # [T, H] -> transpose -> [H, T] -> [KO, 128, T] -> [128, KO, T]
        m["xtd"] = np.ascontiguousarray(
            toks[c * T:(c + 1) * T].T.reshape(KO, 128, T).transpose(1, 0, 2)
            .astype(np.float16)
        )
        in_maps.append(m)
    return in_maps


def run(inputs, trace=False):
    """Returns (full output [B,S,H] fp32, BassKernelResults)."""
    if "nc" not in _CACHE:
        _CACHE["nc"] = _build()
    nc = _CACHE["nc"]
    in_maps = _prep(inputs)
    res = run_bass_kernel_spmd(
        nc, in_maps, core_ids=list(range(N_CORES)), trace=trace
    )
    full = np.empty((NTOK, H), np.float32)
    for c in range(N_CORES):
        # [128, KO, T] -> [KO, 128, T] = [H, T] -> [T, H]
        o = res.results[c]["out"].transpose(1, 0, 2).reshape(H, T)
        full[c * T:(c + 1) * T] = o.T.astype(np.float32)
    return full.reshape(B, S, H), res


def kernel(**inputs):
    full, _ = run(inputs, trace=False)
    return full
